# revision 15
# baseline (speedup 1.0000x reference)
"""Trainium2 kernel for nn_AttShiftW: channel-mean attention-shift weighting.

Pipeline per input tensor [B,C,H,W] = [16,64,128,256] f32:
  1. (device, memory-bound) channel sum over C  -> [B,H,W]
  2. (host, ~6MB of data) /64, per-sample minmax normalize, threshold at 0.4,
     3D 6-connected component labeling across (B,H,W) (scipy), largest region
     per batch slice, masked result, centroid -> spherical distances.

Device sharding: data-parallel over B across 8 cores; each core reduces
2 batches x 3 inputs = 6 slabs of [64,128,256] (48 MB read/core).
The f32 tree-fold order on device (pair c, c+s) is reproduced nowhere on the
host - it doesn't need to be: validated flip-free against the reference on the
fixed inputs, and min/max/threshold arithmetic is exact given the sums.
"""
import numpy as np
from scipy import ndimage

import jax
from jax.sharding import Mesh, PartitionSpec
from jax.experimental.shard_map import shard_map

import concourse.bacc as bacc
import concourse.tile as tile
from concourse import mybir
from concourse import bass2jax

B, C, H, W = 16, 64, 128, 256
N_CORES = 8
B_PER_CORE = B // N_CORES          # 2
N_SLABS = 3 * B_PER_CORE           # 6 slabs per core (3 inputs x 2 batches)
SHIFT_THRESH = 0.4

_CACHE = {}


def _build_bass(repeats=1):
    f32 = mybir.dt.float32
    nc = bacc.Bacc("TRN2")
    xs = [
        nc.declare_dram_parameter(f"input_{t}", [B_PER_CORE, C, H, W], f32, isOutput=False)
        for t in (1, 2, 3)
    ]
    out = nc.declare_dram_parameter("out", [N_SLABS, H, W], f32, isOutput=True)

    with tile.TileContext(nc) as tc:
        with (
            tc.tile_pool(name="x", bufs=2) as xpool,
            tc.tile_pool(name="acc", bufs=2) as apool,
        ):
            for _ in range(repeats):
                # One accumulator strip per sweep; the final fold of each slab
                # writes into it and a single out-DMA ships all 6 slabs.
                # Keeping the input DMAs alone on the SP HWDGE ring matters:
                # an out-DMA interleaved there stalls the FIFO behind the
                # fold dependency (~25us/iter measured), so the output goes
                # out on the idle ACT ring instead.
                acc = apool.tile([H, N_SLABS * W], f32)
                for s in range(N_SLABS):
                    t, b = divmod(s, B_PER_CORE)
                    src = xs[t][b]                      # [C,H,W]
                    xt = xpool.tile([H, C * W], f32)    # [128, 16384]
                    nc.sync.dma_start(
                        xt[:].rearrange("h (c w) -> h c w", c=C),
                        src.rearrange("c h w -> h c w"),
                    )
                    # tree fold over the C axis: block j += block j+step
                    step = C // 2
                    while step >= 2:
                        nc.vector.tensor_add(
                            xt[:, : step * W], xt[:, : step * W], xt[:, step * W : 2 * step * W]
                        )
                        step //= 2
                    nc.vector.tensor_add(
                        acc[:, s * W : (s + 1) * W], xt[:, :W], xt[:, W : 2 * W]
                    )
                nc.scalar.dma_start(
                    out.rearrange("s h w -> h s w"),
                    acc[:].rearrange("h (s w) -> h s w", s=N_SLABS),
                )
    nc.finalize()
    return nc


def _make_sharded(nc):
    """jit-compile a finalized Bass module for SPMD over the 8 cores."""
    bass2jax.install_neuronx_cc_hook()

    partition_name = nc.partition_id_tensor.name if nc.partition_id_tensor else None
    in_names, out_names, out_avals = [], [], []
    for alloc in nc.m.functions[0].allocations:
        if not isinstance(alloc, mybir.MemoryLocationSet):
            continue
        name = alloc.memorylocations[0].name
        if alloc.kind == "ExternalInput":
            if name != partition_name:
                in_names.append(name)
        elif alloc.kind == "ExternalOutput":
            out_names.append(name)
            out_avals.append(
                jax.core.ShapedArray(tuple(alloc.tensor_shape), mybir.dt.np(alloc.dtype))
            )
    assert in_names == ["input_1", "input_2", "input_3"] and out_names == ["out"]
    assert nc.dbg_addr is None
    n_params = len(in_names)
    all_names = list(in_names + out_names)
    if partition_name is not None:
        all_names.append(partition_name)

    def _body(*args):
        operands = list(args)
        if partition_name is not None:
            operands.append(bass2jax.partition_id_tensor())
        return tuple(
            bass2jax._bass_exec_p.bind(
                *operands,
                out_avals=tuple(out_avals),
                in_names=tuple(all_names),
                out_names=tuple(out_names),
                lowering_input_output_aliases=(),
                sim_require_finite=True,
                sim_require_nnan=True,
                nc=nc,
            )
        )

    devices = jax.devices()[:N_CORES]
    mesh = Mesh(np.asarray(devices), ("core",))
    sharded = jax.jit(
        shard_map(
            _body,
            mesh=mesh,
            in_specs=(PartitionSpec("core"),) * (n_params + 1),
            out_specs=(PartitionSpec("core"),),
            check_rep=False,
        ),
        donate_argnums=(n_params,),
        keep_unused=True,
    )
    return sharded, mesh


def _get_runner():
    """Compile once; return f(x1, x2, x3 full arrays) -> out [8, 6, H, W]."""
    if "runner" in _CACHE:
        return _CACHE["runner"]
    nc = _build_bass()
    sharded, mesh = _make_sharded(nc)

    def runner(x1, x2, x3):
        # The kernel writes every output element, so the donated "zero" buffer
        # contents are irrelevant - recycle the previous device output to skip
        # a 6MB host->device upload per call. One retry covers transient
        # device/relay hiccups.
        for attempt in (0, 1):
            zeros = _CACHE.pop("donate_buf", None)
            if zeros is None:
                zeros = np.zeros((N_CORES * N_SLABS, H, W), np.float32)
            try:
                (o,) = sharded(x1, x2, x3, zeros)
                arr = np.asarray(o)
            except Exception:
                if attempt == 1:
                    raise
                continue
            _CACHE["donate_buf"] = o
            return arr.reshape(N_CORES, N_SLABS, H, W)

    _CACHE["runner"] = runner
    _CACHE["mesh"] = mesh
    _CACHE["sharded"] = sharded
    return runner


def _channel_sums(input_1, input_2, input_3):
    """Run the device kernel; returns sums[3, B, H, W] float32 (sum over C)."""
    o = _get_runner()(input_1, input_2, input_3)  # [cores, 6, H, W]
    sums = np.empty((3, B, H, W), dtype=np.float32)
    for i in range(N_CORES):
        for s in range(N_SLABS):
            t, b = divmod(s, B_PER_CORE)
            sums[t, i * B_PER_CORE + b] = o[i, s]
    return sums


def _find_max(csum):
    """csum: [B,H,W] f32 channel sums of one input. Returns theta, phi, result."""
    cmean = (csum / np.float32(C)).astype(np.float32)          # exact /64
    mn = cmean.min(axis=(1, 2), keepdims=True)
    mx = cmean.max(axis=(1, 2), keepdims=True)
    cmean = ((cmean - mn) / (mx - mn)).astype(np.float32)
    maxval = cmean.max(axis=(1, 2), keepdims=True)
    thr = (maxval * np.float32(SHIFT_THRESH)).astype(np.float32)
    mask = cmean >= thr

    # 6-connected labeling over the whole [B,H,W] volume (connects across B),
    # exactly scipy.ndimage.label of the [B,1,H,W] tensor's default structure.
    labels, nlab = ndimage.label(mask)
    # Reference labels each component by its max linear index; argmax over
    # per-slice counts tie-breaks toward the smallest label value. Reproduce:
    # max count wins, ties -> component with smallest max-linear-index.
    lin = np.arange(1, labels.size + 1, dtype=np.int64).reshape(labels.shape)
    comp_maxlin = np.asarray(
        ndimage.maximum(lin, labels, index=np.arange(1, nlab + 1)), dtype=np.int64
    ) if nlab > 0 else np.zeros(0, np.int64)

    big = np.zeros((B, H, W), dtype=bool)
    for i in range(B):
        cnt = np.bincount(labels[i].ravel(), minlength=nlab + 1)[1:]
        if cnt.size == 0 or cnt.max() == 0:
            continue
        cands = np.nonzero(cnt == cnt.max())[0]
        best = cands[np.argmin(comp_maxlin[cands])]
        big[i] = labels[i] == best + 1

    result = np.where(big, cmean, np.float32(0.0))[:, None]    # [B,1,H,W]
    m = big.astype(np.float64)
    cnt = m.sum(axis=(1, 2))
    h_mean = (m * np.arange(H)[:, None]).sum(axis=(1, 2)) / cnt
    w_mean = (m * np.arange(W)[None, :]).sum(axis=(1, 2)) / cnt
    phi = (0.5 - h_mean / H) * np.pi
    theta = (w_mean / W - 0.5) * 2.0 * np.pi
    return theta, phi, result


def _spherical_distance(t1, p1, t2, p2):
    cosd = np.sin(t1) * np.sin(t2) + np.cos(t1) * np.cos(t2) * np.cos(p1 - p2)
    with np.errstate(invalid="ignore"):
        d = np.arccos(cosd) / np.pi
    return np.nan_to_num(d, nan=0.0).astype(np.float32)


def kernel(input_1, input_2, input_3):
    sums = _channel_sums(
        np.asarray(input_1, dtype=np.float32),
        np.asarray(input_2, dtype=np.float32),
        np.asarray(input_3, dtype=np.float32),
    )
    t1, p1, r1 = _find_max(sums[0])
    t2, p2, r2 = _find_max(sums[1])
    t3, p3, r3 = _find_max(sums[2])
    w1 = _spherical_distance(t1, p1, t2, p2)[:, None, None, None]
    w2 = _spherical_distance(t2, p2, t3, p3)[:, None, None, None]
    return w1, w2, r1, r2, r3


# revision 16
# speedup vs baseline: 1.0600x; 1.0600x over previous
"""Trainium2 kernel for nn_AttShiftW: channel-mean attention-shift weighting.

Pipeline per input tensor [B,C,H,W] = [16,64,128,256] f32:
  1. (device, memory-bound) channel sum over C  -> [B,H,W]
  2. (host, ~6MB of data) /64, per-sample minmax normalize, threshold at 0.4,
     3D 6-connected component labeling across (B,H,W) (scipy), largest region
     per batch slice, masked result, centroid -> spherical distances.

Device sharding: data-parallel over B across 8 cores; each core reduces
2 batches x 3 inputs = 6 slabs of [64,128,256] (48 MB read/core).
The f32 tree-fold order on device (pair c, c+s) is reproduced nowhere on the
host - it doesn't need to be: validated flip-free against the reference on the
fixed inputs, and min/max/threshold arithmetic is exact given the sums.
"""
import numpy as np
from scipy import ndimage

import jax
from jax.sharding import Mesh, PartitionSpec
from jax.experimental.shard_map import shard_map

import concourse.bacc as bacc
import concourse.tile as tile
from concourse import mybir
from concourse import bass2jax

B, C, H, W = 16, 64, 128, 256
N_CORES = 8
B_PER_CORE = B // N_CORES          # 2
N_SLABS = 3 * B_PER_CORE           # 6 slabs per core (3 inputs x 2 batches)
SHIFT_THRESH = 0.4

_CACHE = {}


def _build_bass(repeats=1):
    f32 = mybir.dt.float32
    nc = bacc.Bacc("TRN2")
    xs = [
        nc.declare_dram_parameter(f"input_{t}", [B_PER_CORE, C, H, W], f32, isOutput=False)
        for t in (1, 2, 3)
    ]
    out = nc.declare_dram_parameter("out", [N_SLABS, H, W], f32, isOutput=True)

    n_split = 4                        # c-groups per slab; 16-c tiles pipeline
    grp = C // n_split                 # best (release buffers early)
    with tile.TileContext(nc) as tc:
        with (
            tc.tile_pool(name="x", bufs=2 * n_split) as xpool,
            tc.tile_pool(name="acc", bufs=2) as apool,
        ):
            for _ in range(repeats):
                # One accumulator strip per sweep; the final fold of each slab
                # writes into it and a single out-DMA ships all 6 slabs.
                # Keeping the input DMAs alone on the SP HWDGE ring matters:
                # an out-DMA interleaved there stalls the FIFO behind the
                # fold dependency (~25us/iter measured), so the output goes
                # out on the idle ACT ring instead.
                acc = apool.tile([H, N_SLABS * W], f32)
                for s in range(N_SLABS):
                    t, b = divmod(s, B_PER_CORE)
                    src = xs[t][b]                      # [C,H,W]
                    tiles = []
                    for g in range(n_split):
                        xt = xpool.tile([H, grp * W], f32)   # [128, 4096]
                        nc.sync.dma_start(
                            xt[:].rearrange("h (c w) -> h c w", c=grp),
                            src[g * grp : (g + 1) * grp].rearrange("c h w -> h c w"),
                        )
                        tiles.append(xt)
                    # tree fold over the C axis: block j += block j+step,
                    # first across tiles, then within tiles[0]
                    step = C // 2
                    while step >= grp:
                        k = step // grp
                        for j in range(k):
                            nc.vector.tensor_add(tiles[j][:], tiles[j][:], tiles[j + k][:])
                        step //= 2
                    t0 = tiles[0]
                    while step >= 2:
                        nc.vector.tensor_add(
                            t0[:, : step * W], t0[:, : step * W], t0[:, step * W : 2 * step * W]
                        )
                        step //= 2
                    nc.vector.tensor_add(
                        acc[:, s * W : (s + 1) * W], t0[:, :W], t0[:, W : 2 * W]
                    )
                nc.scalar.dma_start(
                    out.rearrange("s h w -> h s w"),
                    acc[:].rearrange("h (s w) -> h s w", s=N_SLABS),
                )
    nc.finalize()
    return nc


def _make_sharded(nc):
    """jit-compile a finalized Bass module for SPMD over the 8 cores."""
    bass2jax.install_neuronx_cc_hook()

    partition_name = nc.partition_id_tensor.name if nc.partition_id_tensor else None
    in_names, out_names, out_avals = [], [], []
    for alloc in nc.m.functions[0].allocations:
        if not isinstance(alloc, mybir.MemoryLocationSet):
            continue
        name = alloc.memorylocations[0].name
        if alloc.kind == "ExternalInput":
            if name != partition_name:
                in_names.append(name)
        elif alloc.kind == "ExternalOutput":
            out_names.append(name)
            out_avals.append(
                jax.core.ShapedArray(tuple(alloc.tensor_shape), mybir.dt.np(alloc.dtype))
            )
    assert in_names == ["input_1", "input_2", "input_3"] and out_names == ["out"]
    assert nc.dbg_addr is None
    n_params = len(in_names)
    all_names = list(in_names + out_names)
    if partition_name is not None:
        all_names.append(partition_name)

    def _body(*args):
        operands = list(args)
        if partition_name is not None:
            operands.append(bass2jax.partition_id_tensor())
        return tuple(
            bass2jax._bass_exec_p.bind(
                *operands,
                out_avals=tuple(out_avals),
                in_names=tuple(all_names),
                out_names=tuple(out_names),
                lowering_input_output_aliases=(),
                sim_require_finite=True,
                sim_require_nnan=True,
                nc=nc,
            )
        )

    devices = jax.devices()[:N_CORES]
    mesh = Mesh(np.asarray(devices), ("core",))
    sharded = jax.jit(
        shard_map(
            _body,
            mesh=mesh,
            in_specs=(PartitionSpec("core"),) * (n_params + 1),
            out_specs=(PartitionSpec("core"),),
            check_rep=False,
        ),
        donate_argnums=(n_params,),
        keep_unused=True,
    )
    return sharded, mesh


def _get_runner():
    """Compile once; return f(x1, x2, x3 full arrays) -> out [8, 6, H, W]."""
    if "runner" in _CACHE:
        return _CACHE["runner"]
    nc = _build_bass()
    sharded, mesh = _make_sharded(nc)

    def runner(x1, x2, x3):
        # The kernel writes every output element, so the donated "zero" buffer
        # contents are irrelevant - recycle the previous device output to skip
        # a 6MB host->device upload per call. One retry covers transient
        # device/relay hiccups.
        for attempt in (0, 1):
            zeros = _CACHE.pop("donate_buf", None)
            if zeros is None:
                zeros = np.zeros((N_CORES * N_SLABS, H, W), np.float32)
            try:
                (o,) = sharded(x1, x2, x3, zeros)
                arr = np.asarray(o)
            except Exception:
                if attempt == 1:
                    raise
                continue
            _CACHE["donate_buf"] = o
            return arr.reshape(N_CORES, N_SLABS, H, W)

    _CACHE["runner"] = runner
    _CACHE["mesh"] = mesh
    _CACHE["sharded"] = sharded
    return runner


def _channel_sums(input_1, input_2, input_3):
    """Run the device kernel; returns sums[3, B, H, W] float32 (sum over C)."""
    o = _get_runner()(input_1, input_2, input_3)  # [cores, 6, H, W]
    sums = np.empty((3, B, H, W), dtype=np.float32)
    for i in range(N_CORES):
        for s in range(N_SLABS):
            t, b = divmod(s, B_PER_CORE)
            sums[t, i * B_PER_CORE + b] = o[i, s]
    return sums


def _find_max(csum):
    """csum: [B,H,W] f32 channel sums of one input. Returns theta, phi, result."""
    cmean = (csum / np.float32(C)).astype(np.float32)          # exact /64
    mn = cmean.min(axis=(1, 2), keepdims=True)
    mx = cmean.max(axis=(1, 2), keepdims=True)
    cmean = ((cmean - mn) / (mx - mn)).astype(np.float32)
    maxval = cmean.max(axis=(1, 2), keepdims=True)
    thr = (maxval * np.float32(SHIFT_THRESH)).astype(np.float32)
    mask = cmean >= thr

    # 6-connected labeling over the whole [B,H,W] volume (connects across B),
    # exactly scipy.ndimage.label of the [B,1,H,W] tensor's default structure.
    labels, nlab = ndimage.label(mask)
    # Reference labels each component by its max linear index; argmax over
    # per-slice counts tie-breaks toward the smallest label value. Reproduce:
    # max count wins, ties -> component with smallest max-linear-index.
    lin = np.arange(1, labels.size + 1, dtype=np.int64).reshape(labels.shape)
    comp_maxlin = np.asarray(
        ndimage.maximum(lin, labels, index=np.arange(1, nlab + 1)), dtype=np.int64
    ) if nlab > 0 else np.zeros(0, np.int64)

    big = np.zeros((B, H, W), dtype=bool)
    for i in range(B):
        cnt = np.bincount(labels[i].ravel(), minlength=nlab + 1)[1:]
        if cnt.size == 0 or cnt.max() == 0:
            continue
        cands = np.nonzero(cnt == cnt.max())[0]
        best = cands[np.argmin(comp_maxlin[cands])]
        big[i] = labels[i] == best + 1

    result = np.where(big, cmean, np.float32(0.0))[:, None]    # [B,1,H,W]
    m = big.astype(np.float64)
    cnt = m.sum(axis=(1, 2))
    h_mean = (m * np.arange(H)[:, None]).sum(axis=(1, 2)) / cnt
    w_mean = (m * np.arange(W)[None, :]).sum(axis=(1, 2)) / cnt
    phi = (0.5 - h_mean / H) * np.pi
    theta = (w_mean / W - 0.5) * 2.0 * np.pi
    return theta, phi, result


def _spherical_distance(t1, p1, t2, p2):
    cosd = np.sin(t1) * np.sin(t2) + np.cos(t1) * np.cos(t2) * np.cos(p1 - p2)
    with np.errstate(invalid="ignore"):
        d = np.arccos(cosd) / np.pi
    return np.nan_to_num(d, nan=0.0).astype(np.float32)


def kernel(input_1, input_2, input_3):
    sums = _channel_sums(
        np.asarray(input_1, dtype=np.float32),
        np.asarray(input_2, dtype=np.float32),
        np.asarray(input_3, dtype=np.float32),
    )
    t1, p1, r1 = _find_max(sums[0])
    t2, p2, r2 = _find_max(sums[1])
    t3, p3, r3 = _find_max(sums[2])
    w1 = _spherical_distance(t1, p1, t2, p2)[:, None, None, None]
    w2 = _spherical_distance(t2, p2, t3, p3)[:, None, None, None]
    return w1, w2, r1, r2, r3


# revision 19
# speedup vs baseline: 1.0936x; 1.0318x over previous
"""Trainium2 kernel for nn_AttShiftW: channel-mean attention-shift weighting.

Pipeline per input tensor [B,C,H,W] = [16,64,128,256] f32:
  1. (device, memory-bound) channel sum over C  -> [B,H,W]
  2. (host, ~6MB of data) /64, per-sample minmax normalize, threshold at 0.4,
     3D 6-connected component labeling across (B,H,W) (scipy), largest region
     per batch slice, masked result, centroid -> spherical distances.

Device sharding: data-parallel over B across 8 cores; each core reduces
2 batches x 3 inputs = 6 slabs of [64,128,256] (48 MB read/core).
The f32 tree-fold order on device (pair c, c+s) is reproduced nowhere on the
host - it doesn't need to be: validated flip-free against the reference on the
fixed inputs, and min/max/threshold arithmetic is exact given the sums.
"""
import time

import numpy as np
from scipy import ndimage

import jax
from jax.sharding import Mesh, PartitionSpec
from jax.experimental.shard_map import shard_map

import concourse.bacc as bacc
import concourse.tile as tile
from concourse import mybir
from concourse import bass2jax

B, C, H, W = 16, 64, 128, 256
N_CORES = 8
B_PER_CORE = B // N_CORES          # 2
N_SLABS = 3 * B_PER_CORE           # 6 slabs per core (3 inputs x 2 batches)
SHIFT_THRESH = 0.4

_CACHE = {}


def _build_bass(repeats=1):
    f32 = mybir.dt.float32
    nc = bacc.Bacc("TRN2")
    xs = [
        nc.declare_dram_parameter(f"input_{t}", [B_PER_CORE, C, H, W], f32, isOutput=False)
        for t in (1, 2, 3)
    ]
    out = nc.declare_dram_parameter("out", [N_SLABS, H, W], f32, isOutput=True)

    n_split = 4                        # c-groups per slab; 16-c tiles pipeline
    grp = C // n_split                 # best (release buffers early)
    with tile.TileContext(nc) as tc:
        with (
            # 10 bufs = 2.5 slabs in flight (160KB/partition) - measured ~4us
            # better than 8; 16KB/partition per tile
            tc.tile_pool(name="x", bufs=10) as xpool,
            tc.tile_pool(name="acc", bufs=2) as apool,
        ):
            for _ in range(repeats):
                # One accumulator strip per sweep; the final fold of each slab
                # writes into it and a single out-DMA ships all 6 slabs.
                # Keeping the input DMAs alone on the SP HWDGE ring matters:
                # an out-DMA interleaved there stalls the FIFO behind the
                # fold dependency (~25us/iter measured), so the output goes
                # out on the idle ACT ring instead.
                acc = apool.tile([H, N_SLABS * W], f32)
                for s in range(N_SLABS):
                    t, b = divmod(s, B_PER_CORE)
                    src = xs[t][b]                      # [C,H,W]
                    tiles = []
                    for g in range(n_split):
                        xt = xpool.tile([H, grp * W], f32)   # [128, 4096]
                        nc.sync.dma_start(
                            xt[:].rearrange("h (c w) -> h c w", c=grp),
                            src[g * grp : (g + 1) * grp].rearrange("c h w -> h c w"),
                        )
                        tiles.append(xt)
                    # tree fold over the C axis: block j += block j+step,
                    # first across tiles, then within tiles[0]
                    step = C // 2
                    while step >= grp:
                        k = step // grp
                        for j in range(k):
                            nc.vector.tensor_add(tiles[j][:], tiles[j][:], tiles[j + k][:])
                        step //= 2
                    t0 = tiles[0]
                    while step >= 2:
                        nc.vector.tensor_add(
                            t0[:, : step * W], t0[:, : step * W], t0[:, step * W : 2 * step * W]
                        )
                        step //= 2
                    nc.vector.tensor_add(
                        acc[:, s * W : (s + 1) * W], t0[:, :W], t0[:, W : 2 * W]
                    )
                nc.scalar.dma_start(
                    out.rearrange("s h w -> h s w"),
                    acc[:].rearrange("h (s w) -> h s w", s=N_SLABS),
                )
    nc.finalize()
    return nc


def _make_sharded(nc):
    """jit-compile a finalized Bass module for SPMD over the 8 cores."""
    bass2jax.install_neuronx_cc_hook()

    partition_name = nc.partition_id_tensor.name if nc.partition_id_tensor else None
    in_names, out_names, out_avals = [], [], []
    for alloc in nc.m.functions[0].allocations:
        if not isinstance(alloc, mybir.MemoryLocationSet):
            continue
        name = alloc.memorylocations[0].name
        if alloc.kind == "ExternalInput":
            if name != partition_name:
                in_names.append(name)
        elif alloc.kind == "ExternalOutput":
            out_names.append(name)
            out_avals.append(
                jax.core.ShapedArray(tuple(alloc.tensor_shape), mybir.dt.np(alloc.dtype))
            )
    assert in_names == ["input_1", "input_2", "input_3"] and out_names == ["out"]
    assert nc.dbg_addr is None
    n_params = len(in_names)
    all_names = list(in_names + out_names)
    if partition_name is not None:
        all_names.append(partition_name)

    def _body(*args):
        operands = list(args)
        if partition_name is not None:
            operands.append(bass2jax.partition_id_tensor())
        return tuple(
            bass2jax._bass_exec_p.bind(
                *operands,
                out_avals=tuple(out_avals),
                in_names=tuple(all_names),
                out_names=tuple(out_names),
                lowering_input_output_aliases=(),
                sim_require_finite=True,
                sim_require_nnan=True,
                nc=nc,
            )
        )

    devices = jax.devices()[:N_CORES]
    mesh = Mesh(np.asarray(devices), ("core",))
    sharded = jax.jit(
        shard_map(
            _body,
            mesh=mesh,
            in_specs=(PartitionSpec("core"),) * (n_params + 1),
            out_specs=(PartitionSpec("core"),),
            check_rep=False,
        ),
        donate_argnums=(n_params,),
        keep_unused=True,
    )
    return sharded, mesh


def _get_runner():
    """Compile once; return f(x1, x2, x3 full arrays) -> out [8, 6, H, W]."""
    if "runner" in _CACHE:
        return _CACHE["runner"]
    nc = _build_bass()
    sharded, mesh = _make_sharded(nc)

    def runner(x1, x2, x3):
        # The kernel writes every output element, so the donated "zero" buffer
        # contents are irrelevant - recycle the previous device output to skip
        # a 6MB host->device upload per call. Retries cover transient
        # device/relay hiccups (a wedged core recovers within ~30s).
        for attempt in range(3):
            zeros = _CACHE.pop("donate_buf", None)
            if zeros is None:
                zeros = np.zeros((N_CORES * N_SLABS, H, W), np.float32)
            try:
                (o,) = sharded(x1, x2, x3, zeros)
                arr = np.asarray(o)
            except Exception:
                if attempt == 2:
                    raise
                time.sleep(15 * (attempt + 1))
                continue
            _CACHE["donate_buf"] = o
            return arr.reshape(N_CORES, N_SLABS, H, W)

    _CACHE["runner"] = runner
    _CACHE["mesh"] = mesh
    _CACHE["sharded"] = sharded
    return runner


def _channel_sums(input_1, input_2, input_3):
    """Run the device kernel; returns sums[3, B, H, W] float32 (sum over C)."""
    o = _get_runner()(input_1, input_2, input_3)  # [cores, 6, H, W]
    sums = np.empty((3, B, H, W), dtype=np.float32)
    for i in range(N_CORES):
        for s in range(N_SLABS):
            t, b = divmod(s, B_PER_CORE)
            sums[t, i * B_PER_CORE + b] = o[i, s]
    return sums


def _find_max(csum):
    """csum: [B,H,W] f32 channel sums of one input. Returns theta, phi, result."""
    cmean = (csum / np.float32(C)).astype(np.float32)          # exact /64
    mn = cmean.min(axis=(1, 2), keepdims=True)
    mx = cmean.max(axis=(1, 2), keepdims=True)
    cmean = ((cmean - mn) / (mx - mn)).astype(np.float32)
    maxval = cmean.max(axis=(1, 2), keepdims=True)
    thr = (maxval * np.float32(SHIFT_THRESH)).astype(np.float32)
    mask = cmean >= thr

    # 6-connected labeling over the whole [B,H,W] volume (connects across B),
    # exactly scipy.ndimage.label of the [B,1,H,W] tensor's default structure.
    labels, nlab = ndimage.label(mask)
    # Reference labels each component by its max linear index; argmax over
    # per-slice counts tie-breaks toward the smallest label value. Reproduce:
    # max count wins, ties -> component with smallest max-linear-index.
    lin = np.arange(1, labels.size + 1, dtype=np.int64).reshape(labels.shape)
    comp_maxlin = np.asarray(
        ndimage.maximum(lin, labels, index=np.arange(1, nlab + 1)), dtype=np.int64
    ) if nlab > 0 else np.zeros(0, np.int64)

    big = np.zeros((B, H, W), dtype=bool)
    for i in range(B):
        cnt = np.bincount(labels[i].ravel(), minlength=nlab + 1)[1:]
        if cnt.size == 0 or cnt.max() == 0:
            continue
        cands = np.nonzero(cnt == cnt.max())[0]
        best = cands[np.argmin(comp_maxlin[cands])]
        big[i] = labels[i] == best + 1

    result = np.where(big, cmean, np.float32(0.0))[:, None]    # [B,1,H,W]
    m = big.astype(np.float64)
    cnt = m.sum(axis=(1, 2))
    h_mean = (m * np.arange(H)[:, None]).sum(axis=(1, 2)) / cnt
    w_mean = (m * np.arange(W)[None, :]).sum(axis=(1, 2)) / cnt
    phi = (0.5 - h_mean / H) * np.pi
    theta = (w_mean / W - 0.5) * 2.0 * np.pi
    return theta, phi, result


def _spherical_distance(t1, p1, t2, p2):
    cosd = np.sin(t1) * np.sin(t2) + np.cos(t1) * np.cos(t2) * np.cos(p1 - p2)
    with np.errstate(invalid="ignore"):
        d = np.arccos(cosd) / np.pi
    return np.nan_to_num(d, nan=0.0).astype(np.float32)


def kernel(input_1, input_2, input_3):
    sums = _channel_sums(
        np.asarray(input_1, dtype=np.float32),
        np.asarray(input_2, dtype=np.float32),
        np.asarray(input_3, dtype=np.float32),
    )
    t1, p1, r1 = _find_max(sums[0])
    t2, p2, r2 = _find_max(sums[1])
    t3, p3, r3 = _find_max(sums[2])
    w1 = _spherical_distance(t1, p1, t2, p2)[:, None, None, None]
    w2 = _spherical_distance(t2, p2, t3, p3)[:, None, None, None]
    return w1, w2, r1, r2, r3
